# revision 57
# baseline (speedup 1.0000x reference)
"""BlockTransformerPairBias Trainium2 kernel.

Sharding: 8 cores = (batch 0/1) x (4 groups of 16 attention blocks).
Each core computes its 1024 tokens end-to-end; no collectives.
Host slices inputs, folds LN/scale constants into weights, pre-transposes
the framepair slab to [cz, pairs] bf16, and concatenates core outputs.
"""

import sys

sys.path.insert(0, "/opt/trn_rl_repo")

from contextlib import ExitStack

import numpy as np
import ml_dtypes

import concourse.bass as bass
import concourse.tile as tile
from concourse import bacc, mybir
from concourse.bass_utils import run_bass_kernel_spmd
from concourse.masks import make_identity

F32 = mybir.dt.float32
BF16 = mybir.dt.bfloat16
FP8 = mybir.dt.float8e4
I16 = mybir.dt.int16
AF = mybir.ActivationFunctionType
ALU = mybir.AluOpType
BF = ml_dtypes.bfloat16

B, N, NRES = 2, 4096, 1024
CS, CC, CZ, H, BLK = 512, 384, 128, 8, 64
CH = CS // H          # 64
NB = N // BLK         # 64
NCORES = 8
NBLK = NB * B // NCORES   # 16 blocks per core
NT = NBLK * BLK           # 1024 tokens per core
RT = NT // 128            # 8 token tiles
EPS = 1e-5

_CACHE = {}


def _declare(nc):
    t = {}

    def inp(name, shape, dt):
        t[name] = nc.dram_tensor(name, list(shape), dt, kind="ExternalInput").ap()

    inp("re", (NT, CS), F32)
    inp("zT", (NBLK, CZ, BLK * BLK), BF16)
    inp("s", (NRES, CC), F32)
    inp("idx", (128, NT // 16), I16)
    inp("wq", (128, 4, CS), BF16)
    inp("wk", (128, 4, CS), BF16)
    inp("wv", (128, 4, CS), BF16)
    inp("wg", (128, 4, CS), BF16)
    inp("wout", (128, 4, CS), BF16)
    inp("w1", (128, 4, 2 * CS), BF16)
    inp("w2", (128, 4, 2 * CS), BF16)
    inp("wb", (128, 8, CS), BF16)
    inp("wada", (128, 3, 3 * CS), BF16)
    inp("wbs", (CZ, 64), BF16)
    inp("svec", (H,), F32)          # holds MINUS S[h]
    inp("bq", (128, 4), F32)
    inp("bk", (128, 4), F32)
    inp("bada", (3 * CS,), F32)
    t["out"] = nc.dram_tensor("out", [NT, CS], F32, kind="ExternalOutput").ap()
    return t


def _bcast(ap, p=128):
    """Broadcast a 1-D DRAM AP across p partitions."""
    return bass.AP(tensor=ap.tensor, offset=ap.offset, ap=[[0, p]] + list(ap.ap))


def _ln_rstd(nc, sb, eps_t, x_ap):
    """bn stats over free dim -> (mean [P,1], rstd [P,1]) tiles."""
    p = x_ap.shape[0]
    stats = sb.tile([128, 6], F32, tag="stats")
    nc.vector.bn_stats(stats[:p], x_ap)
    mv = sb.tile([128, 2], F32, tag="mv")
    nc.vector.bn_aggr(mv[:p], stats[:p])
    sd = sb.tile([128, 1], F32, tag="sd")
    nc.scalar.activation(sd[:p], mv[:p, 1:2], AF.Sqrt, bias=eps_t[:p], scale=1.0)
    nc.vector.reciprocal(sd[:p], sd[:p])
    return mv[:p, 0:1], sd[:p]


def _ln_stats_batch(nc, sb, eps_t, x_aps, tag):
    """LN stats for several tiles; one Rsqrt table-load for the batch."""
    n = len(x_aps)
    mv = sb.tile([128, n, 2], F32, tag=tag + "mv")
    for i, x in enumerate(x_aps):
        stats = sb.tile([128, 6], F32, tag="stats")
        nc.vector.bn_stats(stats[:], x)
        nc.vector.bn_aggr(mv[:, i, :], stats[:])
    rst = sb.tile([128, n], F32, tag=tag + "rs")
    nc.scalar.activation(rst[:], mv[:, :, 1], AF.Sqrt, bias=eps_t[:], scale=1.0)
    nc.vector.reciprocal(rst[:], rst[:])
    return mv, rst


def _emit(ctx, tc, t, flags):
    nc = tc.nc
    has_bq, has_bk, has_bag, has_bab, has_btg = flags

    consts = ctx.enter_context(tc.tile_pool(name="consts", bufs=1))
    sb = ctx.enter_context(tc.tile_pool(name="sb", bufs=2))
    ps_pt = ctx.enter_context(tc.tile_pool(name="ps_pt", bufs=1, space="PSUM"))
    ps_tp = ctx.enter_context(tc.tile_pool(name="ps_tp", bufs=2, space="PSUM"))
    aa = ctx.enter_context(tc.tile_pool(name="acts", bufs=1))
    # right-side stack: released mid-kernel, independent of the left stack
    xnp = tc.alloc_tile_pool(name="xnp", bufs=1, side="right")

    # ---- constants / weights resident all kernel ----
    ident = consts.tile([128, 128], BF16)
    make_identity(nc, ident[:])
    eps_t = consts.tile([128, 1], F32)
    nc.vector.memset(eps_t[:], EPS)
    ones64 = consts.tile([64, 64], BF16)
    nc.vector.memset(ones64[:], 1.0)

    # re first: the LN/transpose pipeline below is the first consumer
    re = aa.tile([128, RT, CS], F32)
    nc.sync.dma_start(re[:], t["re"].rearrange("(r p) c -> p r c", p=128))

    wbs_sb = consts.tile([CZ, 64], BF16)
    nc.sync.dma_start(wbs_sb[:], t["wbs"][:])
    wbs8 = consts.tile([CZ, 32], FP8)  # z^2-pass cols (1/CZ exact in e4m3)
    nc.vector.tensor_copy(wbs8[:], wbs_sb[:, 32:64])
    svec_sb = consts.tile([128, H], F32)
    nc.sync.dma_start(svec_sb[:], _bcast(t["svec"]))
    idx_sb = consts.tile([128, NT // 16], I16)
    nc.sync.dma_start(idx_sb[:], t["idx"][:])
    bq_sb = consts.tile([128, 4], F32)
    bk_sb = consts.tile([128, 4], F32)
    if has_bq:
        nc.sync.dma_start(bq_sb[:], t["bq"][:])
    if has_bk:
        nc.sync.dma_start(bk_sb[:], t["bk"][:])
    # attention weights: tiles here, DMAs emitted after P1 starts (their
    # transfers overlap P1 compute; s/wada win the DMA ring first)
    wq = consts.tile([128, 4, CS], BF16)
    wk = consts.tile([128, 4, CS], BF16)
    wv = consts.tile([128, 4, CS], BF16)
    wg = consts.tile([128, 4, CS], BF16)
    wout = consts.tile([128, 4, CS], BF16)

    h_sb = re  # h overwrites re in place per pair (residual add)

    dramp = ctx.enter_context(tc.tile_pool(name="dram", bufs=1, space="DRAM"))
    tbl = dramp.tile([NRES, 3 * CS], BF16)

    # ---- LN1 + transpose of rigids_embed (independent of P1; fills startup) ----
    xnT = xnp.tile([128, 4, NT], BF16)
    mv_x, rs_x = _ln_stats_batch(nc, sb, eps_t,
                                 [re[:, r, :] for r in range(RT)], "lnx")
    for r in range(RT):
        xn = sb.tile([128, CS], BF16, tag="xn")
        nc.vector.tensor_scalar(out=xn[:], in0=re[:, r, :],
                                scalar1=mv_x[:, r, 0:1], scalar2=rs_x[:, r:r + 1],
                                op0=ALU.subtract, op1=ALU.mult)
        for c in range(4):
            tp = ps_tp.tile([128, 128], BF16, tag="tp")
            nc.tensor.transpose(tp[:], xn[:, c * 128:(c + 1) * 128], ident[:])
            nc.scalar.copy(xnT[:, c, r * 128:(r + 1) * 128], tp[:])

    # ================= P1: cond tables =================
    with tc.tile_pool(name="p1", bufs=2) as p1p, \
         tc.tile_pool(name="ps_p1", bufs=3, space="PSUM") as ps_p1:
        wada = None
        with tc.tile_pool(name="p1w", bufs=1) as p1w:
            wada = p1w.tile([128, 3, 3 * CS], BF16)
            nc.sync.dma_start(wada[:], t["wada"][:])
            bada_bc = p1w.tile([128, 3 * CS], F32)
            if has_bag or has_bab or has_btg:
                nc.sync.dma_start(bada_bc[:], _bcast(t["bada"]))
            s_all = p1w.tile([128, NRES // 128, CC], F32)
            nc.sync.dma_start(
                s_all[:], t["s"].rearrange("(r p) c -> p r c", p=128))
            # attention weights ride the scalar queue's DMA ring; transfers
            # overlap P1 compute without delaying s/wada on the sync ring
            nc.scalar.dma_start(wq[:], t["wq"][:])
            nc.scalar.dma_start(wk[:], t["wk"][:])
            nc.scalar.dma_start(wv[:], t["wv"][:])
            nc.scalar.dma_start(wg[:], t["wg"][:])
            nc.scalar.dma_start(wout[:], t["wout"][:])
            mv_s, rs_s = _ln_stats_batch(
                nc, sb, eps_t,
                [s_all[:, r, :] for r in range(NRES // 128)], "lns")
            for r in range(NRES // 128):
                cond = p1p.tile([128, CC], BF16, tag="cond")
                nc.vector.tensor_scalar(out=cond[:], in0=s_all[:, r, :],
                                        scalar1=mv_s[:, r, 0:1],
                                        scalar2=rs_s[:, r:r + 1],
                                        op0=ALU.subtract, op1=ALU.mult)
                ct = p1p.tile([128, 3, 128], BF16, tag="ct")
                for c in range(3):
                    tp = ps_tp.tile([128, 128], BF16, tag="tp")
                    nc.tensor.transpose(tp[:], cond[:, c * 128:(c + 1) * 128], ident[:])
                    nc.vector.tensor_copy(ct[:, c, :], tp[:])
                tbl_sb = p1p.tile([128, 3 * CS], BF16, tag="tbl_sb")
                for n in range(3):
                    pt = ps_p1.tile([128, CS], F32, tag="pt")
                    for k in range(3):
                        nc.tensor.matmul(pt[:], ct[:, k, :],
                                         wada[:, k, n * CS:(n + 1) * CS],
                                         start=(k == 0), stop=(k == 2))
                    seg = slice(n * CS, (n + 1) * CS)
                    if n == 0:
                        if has_bag:
                            nc.vector.tensor_add(pt[:], pt[:], bada_bc[:, seg])
                        nc.scalar.activation(tbl_sb[:, seg], pt[:], AF.Sigmoid)
                    elif n == 1:
                        if has_bab:
                            nc.vector.tensor_add(tbl_sb[:, seg], pt[:], bada_bc[:, seg])
                        else:
                            nc.vector.tensor_copy(tbl_sb[:, seg], pt[:])
                    else:
                        if has_btg:
                            nc.vector.tensor_add(pt[:], pt[:], bada_bc[:, seg])
                        nc.scalar.activation(tbl_sb[:, seg], pt[:], AF.Sigmoid)
                nc.gpsimd.dma_start(tbl[r * 128:(r + 1) * 128, :], tbl_sb[:])

    # gather result + transition weights: allocated early so the gathers and
    # weight DMAs overlap the attention phase
    gth = aa.tile([128, RT, 3 * CS], BF16)
    w1 = aa.tile([128, 4, 2 * CS], BF16)
    nc.scalar.dma_start(w1[:], t["w1"][:])
    w2 = aa.tile([128, 4, 2 * CS], BF16)
    nc.scalar.dma_start(w2[:], t["w2"][:])
    wb = aa.tile([128, 8, CS], BF16)
    nc.scalar.dma_start(wb[:], t["wb"][:])

    # ============ P3..P7: projections, bias path, attention ============
    with tc.tile_pool(name="att2", bufs=1) as at2:
        # ---- projections ----
        qf = at2.tile([128, 4, NT], BF16)
        kf = at2.tile([128, 4, NT], BF16)
        for (w, bias_sb, has_b, dst) in ((wq, bq_sb, has_bq, qf),
                                         (wk, bk_sb, has_bk, kf)):
            for m in range(4):
                for n in range(2):
                    pt = ps_pt.tile([128, CS], F32, tag="pt")
                    for k in range(4):
                        nc.tensor.matmul(pt[:], w[:, k, m * 128:(m + 1) * 128],
                                         xnT[:, k, n * 512:(n + 1) * 512],
                                         start=(k == 0), stop=(k == 3))
                    dseg = dst[:, m, n * 512:(n + 1) * 512]
                    if has_b:
                        nc.vector.tensor_scalar_add(out=dseg, in0=pt[:],
                                                    scalar1=bias_sb[:, m:m + 1])
                    else:
                        nc.vector.tensor_copy(dseg, pt[:])
        # odd heads' q/k rows duplicated at partition base 0: every QK matmul
        # then issues from PE row-group 0 (mixed row-groups draining into one
        # PSUM bank concurrently crash the device)
        qf2 = at2.tile([64, 4, NT], BF16)
        nc.sync.dma_start(qf2[:], qf[64:128, :, :])
        kf2 = at2.tile([64, 4, NT], BF16)
        nc.sync.dma_start(kf2[:], kf[64:128, :, :])
        vtm = at2.tile([64, NBLK, CS], BF16)
        gsigT = at2.tile([128, 4, NT], BF16)
        for r in range(RT):
            pt = ps_pt.tile([128, CS], F32, tag="pt")
            for k in range(4):
                nc.tensor.matmul(pt[:], xnT[:, k, r * 128:(r + 1) * 128],
                                 wv[:, k, :], start=(k == 0), stop=(k == 3))
            # rows 0:64 = block 2r, rows 64:128 = block 2r+1 (DMA repartitions)
            nc.vector.tensor_copy(vtm[:, 2 * r, :], pt[0:64, :])
            vst = sb.tile([128, CS], BF16, tag="vst")
            nc.vector.tensor_copy(vst[64:128, :], pt[64:128, :])
            nc.sync.dma_start(vtm[:, 2 * r + 1, :], vst[64:128, :])
        # gate, transposed: gsigT[c, tokens] feeds the o^T elementwise gate
        for m in range(4):
            for n in range(2):
                pt2 = ps_pt.tile([128, CS], F32, tag="pt")
                for k in range(4):
                    nc.tensor.matmul(pt2[:], wg[:, k, m * 128:(m + 1) * 128],
                                     xnT[:, k, n * 512:(n + 1) * 512],
                                     start=(k == 0), stop=(k == 3))
                nc.scalar.activation(gsigT[:, m, n * 512:(n + 1) * 512], pt2[:],
                                     AF.Sigmoid)
        xnp.release()

        # ---- bias path + attention + Wout, per block pair ----
        from concourse.tile import add_dep_helper
        with tc.tile_pool(name="big", bufs=2) as big, \
             tc.tile_pool(name="z2p", bufs=2) as z2p, \
             tc.tile_pool(name="dramP", bufs=2, space="DRAM") as dpp, \
             tc.tile_pool(name="ps_pz", bufs=2, space="PSUM") as ps_pz, \
             tc.tile_pool(name="ps_sc", bufs=1, space="PSUM") as ps_sc, \
             tc.tile_pool(name="ps_rs", bufs=1, space="PSUM") as ps_rs, \
             tc.tile_pool(name="ps_ot", bufs=1, space="PSUM") as ps_ot:
            prev_lds = {}
            for gp in range(RT):
                Pr = sb.tile([128, 10, 64], BF16, tag="Pr")
                zts, z2s = [], []
                for g2 in range(2):
                    zt = big.tile([CZ, BLK * BLK], BF16, tag="zt")
                    nc.sync.dma_start(zt[:], t["zT"][2 * gp + g2])
                    zts.append(zt)
                for g2 in range(2):
                    z2 = z2p.tile([CZ, BLK * BLK], FP8, tag="z2")
                    nc.scalar.activation(z2[:], zts[g2][:], AF.Square)
                    z2s.append(z2)
                for g2 in range(2):
                    g = 2 * gp + g2
                    zt, z2 = zts[g2], z2s[g2]
                    Psbb = big.tile([128, 1024], BF16, tag="Psbb")
                    ze = ps_pz.tile([128, 512], F32, tag="pz")
                    zo = ps_pz.tile([128, 512], F32, tag="pz")
                    for cg in range(4):
                        tpos = (0, 32 * cg)
                        rows = slice(32 * cg, 32 * cg + 32)
                        ev = slice((2 * cg) * 512, (2 * cg + 1) * 512)
                        od = slice((2 * cg + 1) * 512, (2 * cg + 2) * 512)
                        # z pass writes P rows 0..8; z^2 pass accumulates into
                        # row 9 via a shifted ones column (start=False).
                        nc.tensor.matmul(ze[rows, :], wbs_sb[:, 0:32], zt[:, ev],
                                         start=True, stop=False, tile_position=tpos)
                        nc.tensor.matmul(zo[rows, :], wbs_sb[:, 0:32], zt[:, od],
                                         start=True, stop=False, tile_position=tpos)
                        nc.tensor.matmul(ze[rows, :], wbs8[:], z2[:, ev],
                                         start=False, stop=True, tile_position=tpos)
                        nc.tensor.matmul(zo[rows, :], wbs8[:], z2[:, od],
                                         start=False, stop=True, tile_position=tpos)
                    nc.scalar.copy(Psbb[:, 0:512], ze[:])
                    nc.vector.tensor_copy(Psbb[:, 512:1024], zo[:])
                    # round-trip through DRAM to reshape [32cg+m, (ab i3 j)]
                    # -> [i=(cg ab i3), m, j]
                    dP = dpp.tile([128, 1024], BF16, tag="dP")
                    st = nc.gpsimd.dma_start(dP[:], Psbb[:])
                    for l in prev_lds.get(g % 2, ()):
                        add_dep_helper(st.ins, l, reason="dramP WAR")
                    # gather lands Pr[j, m, i]: the host enumerates z pairs
                    # j-major, so this reshape directly yields the S^T-layout
                    # bias (j on partitions) -- no on-chip transpose needed
                    base = dP[:]
                    lds = []
                    for cg in range(4):
                        src = bass.AP(tensor=base.tensor,
                                      offset=base.offset + cg * 32768,
                                      ap=[[64, 16], [1024, 10], [1, 64]])
                        ld = nc.gpsimd.dma_start(
                            Pr[g2 * 64 + cg * 16:g2 * 64 + (cg + 1) * 16, :, :],
                            src)
                        add_dep_helper(ld.ins, st.ins, reason="reshape RAW")
                        lds.append(ld.ins)
                    prev_lds[g % 2] = lds

                # stats for the pair: mean in Pr[:,8], E[z^2] in Pr[:,9]
                msq = sb.tile([128, 64], F32, tag="msq")
                nc.vector.tensor_mul(msq[:], Pr[:, 8, :], Pr[:, 8, :])
                var_t = sb.tile([128, 64], F32, tag="var_t")
                nc.vector.tensor_sub(var_t[:], Pr[:, 9, :], msq[:])
                nc.scalar.activation(var_t[:], var_t[:], AF.Sqrt,
                                     bias=eps_t[:], scale=1.0)
                rstd_t = sb.tile([128, 64], F32, tag="rstd_t")
                nc.vector.reciprocal(rstd_t[:], var_t[:])
                mr_t = sb.tile([128, 64], F32, tag="mr_t")
                nc.vector.tensor_mul(mr_t[:], Pr[:, 8, :], rstd_t[:])

                def b0(ap_, reps, at=None):
                    lst = list(ap_.ap)
                    pos = len(lst) if at is None else at
                    lst.insert(pos, [0, reps])
                    return bass.AP(tensor=ap_.tensor, offset=ap_.offset, ap=lst)

                # bias_all[p,(h,j)] = Pr_h*rstd - S_h*mean*rstd  (svec = -S)
                mrs = sb.tile([128, H, 64], BF16, tag="mrs")
                nc.vector.tensor_mul(mrs[:], b0(mr_t[:], H, at=1), b0(svec_sb[:], 64))
                bias_all = sb.tile([128, H, 64], F32, tag="bias_all")
                nc.vector.tensor_mul(bias_all[:], Pr[:, 0:H, :],
                                     b0(rstd_t[:], H, at=1))
                nc.vector.tensor_add(bias_all[:], bias_all[:], mrs[:])

                # ---- attention, S^T layout: partitions=j, free=(h,i) ----
                sc_ps = ps_sc.tile([128, CS], F32, tag="sc_ps")
                for g2 in range(2):
                    g = 2 * gp + g2
                    for h in range(H):
                        m = h // 2
                        qsl = (qf[0:64, m, g * 64:(g + 1) * 64] if h % 2 == 0
                               else qf2[:, m, g * 64:(g + 1) * 64])
                        ksl = (kf[0:64, m, g * 64:(g + 1) * 64] if h % 2 == 0
                               else kf2[:, m, g * 64:(g + 1) * 64])
                        nc.tensor.matmul(sc_ps[g2 * 64:g2 * 64 + 64,
                                               h * 64:(h + 1) * 64],
                                         ksl, qsl, start=True, stop=True,
                                         tile_position=(0, g2 * 64))
                sc_sb = sb.tile([128, CS], F32, tag="sc_sb")
                nc.vector.tensor_add(sc_sb[:].rearrange("p (h j) -> p h j", h=H),
                                     sc_ps[:].rearrange("p (h j) -> p h j", h=H),
                                     bias_all[:])
                a_sb = sb.tile([128, CS], BF16, tag="a_sb")
                nc.scalar.activation(a_sb[:], sc_sb[:], AF.Exp)
                # block1 rows copied to partition base 0 (matmul operands must
                # issue from PE row-group 0)
                a1t = sb.tile([64, CS], BF16, tag="a1t")
                nc.sync.dma_start(a1t[:], a_sb[64:128, :])
                # rowsums over j via all-ones stationary: every output row of
                # the 64-wide result holds the sums -> partition-aligned rcp
                rs_ps = ps_rs.tile([128, CS], F32, tag="rs_ps")
                nc.tensor.matmul(rs_ps[0:64, :], ones64[:], a_sb[0:64, :],
                                 start=True, stop=True, tile_position=(0, 0))
                nc.tensor.matmul(rs_ps[64:128, :], ones64[:], a1t[:],
                                 start=True, stop=True, tile_position=(0, 64))
                rcp_sb = sb.tile([128, CS], BF16, tag="rcp_sb")
                with nc.allow_low_precision(reason="softmax norm tolerates bf16"):
                    nc.vector.reciprocal(rcp_sb[:], rs_ps[:])
                rcp1t = sb.tile([64, CS], BF16, tag="rcp1t")
                nc.sync.dma_start(rcp1t[:], rcp_sb[64:128, :])
                a_n = sb.tile([64, 2, CS], BF16, tag="a_n")
                nc.vector.tensor_mul(a_n[:, 0, :], a_sb[0:64, :], rcp_sb[0:64, :])
                nc.vector.tensor_mul(a_n[:, 1, :], a1t[:], rcp1t[:])

                # ---- A@V in transposed form: o^T[c, i] per (g2, h) ----
                oT_ps = ps_ot.tile([128, 4, 128], F32, tag="oT_ps")
                for g2 in range(2):
                    g = 2 * gp + g2
                    for h in range(H):
                        nc.tensor.matmul(
                            oT_ps[(h % 2) * 64:(h % 2) * 64 + 64, h // 2,
                                  g2 * 64:g2 * 64 + 64],
                            vtm[:, g, h * 64:(h + 1) * 64],
                            a_n[:, g2, h * 64:(h + 1) * 64],
                            start=True, stop=True, tile_position=(0, (h % 2) * 64))
                ogT = sb.tile([128, 4, 128], BF16, tag="ogT")
                nc.vector.tensor_mul(ogT[:], oT_ps[:],
                                     gsigT[:, :, gp * 128:(gp + 1) * 128])
                # ---- Wout + residual ----
                pt = ps_pt.tile([128, CS], F32, tag="pt")
                for k in range(4):
                    nc.tensor.matmul(pt[:], ogT[:, k, :], wout[:, k, :],
                                     start=(k == 0), stop=(k == 3))
                nc.vector.tensor_add(h_sb[:, gp, :], pt[:], re[:, gp, :])

                # overlap the cond-table gather with the attention phase
                nc.gpsimd.dma_gather(
                    out_ap=gth[:, gp:gp + 1, :], in_ap=tbl[:],
                    idxs_ap=idx_sb[:, gp * 8:(gp + 1) * 8],
                    num_idxs=128, num_idxs_reg=128, elem_size=3 * CS)

    # ================= P8..P10: transition =================
    with tc.tile_pool(name="acts2", bufs=1) as a2, \
         tc.tile_pool(name="sb2", bufs=2) as sb2, \
         tc.tile_pool(name="ps_tr", bufs=3, space="PSUM") as ps_tr:
        tT = a2.tile([128, 4, NT], BF16)
        for r in range(RT):
            mean, rstd = _ln_rstd(nc, sb, eps_t, h_sb[:, r, :])
            t0 = sb2.tile([128, CS], BF16, tag="t0")
            nc.vector.tensor_scalar(out=t0[:], in0=h_sb[:, r, :], scalar1=mean,
                                    scalar2=rstd, op0=ALU.subtract, op1=ALU.mult)
            t1 = sb2.tile([128, CS], BF16, tag="t1")
            nc.vector.tensor_mul(t1[:], t0[:], gth[:, r, 0:CS])
            t2 = sb2.tile([128, CS], BF16, tag="t2")
            nc.vector.tensor_add(t2[:], t1[:], gth[:, r, CS:2 * CS])
            for c in range(4):
                tp = ps_tp.tile([128, 128], BF16, tag="tp")
                nc.tensor.transpose(tp[:], t2[:, c * 128:(c + 1) * 128], ident[:])
                nc.scalar.copy(tT[:, c, r * 128:(r + 1) * 128], tp[:])

        bb = a2.tile([128, 8, NT], BF16)
        for n in range(2):
            for m in range(8):
                p1 = ps_tr.tile([128, CS], F32, tag="pt")
                for k in range(4):
                    nc.tensor.matmul(p1[:], w1[:, k, m * 128:(m + 1) * 128],
                                     tT[:, k, n * 512:(n + 1) * 512],
                                     start=(k == 0), stop=(k == 3))
                u1s = sb2.tile([128, 512], F32, tag="u1s")
                nc.scalar.activation(u1s[:], p1[:], AF.Sigmoid)
                u1 = sb2.tile([128, 512], F32, tag="u1")
                nc.vector.tensor_mul(u1[:], u1s[:], p1[:])
                p2 = ps_tr.tile([128, CS], F32, tag="pt")
                for k in range(4):
                    nc.tensor.matmul(p2[:], w2[:, k, m * 128:(m + 1) * 128],
                                     tT[:, k, n * 512:(n + 1) * 512],
                                     start=(k == 0), stop=(k == 3))
                nc.vector.tensor_mul(bb[:, m, n * 512:(n + 1) * 512], u1[:], p2[:])
            # this n-chunk of bb is complete: drain its Wb tiles now so the
            # tail overlaps the other chunk's W1/W2 work
            for r in range(4 * n, 4 * n + 4):
                pt = ps_tr.tile([128, CS], F32, tag="pt")
                for k in range(8):
                    nc.tensor.matmul(pt[:], bb[:, k, r * 128:(r + 1) * 128],
                                     wb[:, k, :], start=(k == 0), stop=(k == 7))
                tg32 = sb2.tile([128, CS], F32, tag="tg32")
                nc.scalar.copy(tg32[:], gth[:, r, 2 * CS:3 * CS])
                tr = sb2.tile([128, CS], F32, tag="tr")
                nc.vector.tensor_mul(tr[:], pt[:], tg32[:])
                out_t = sb2.tile([128, CS], F32, tag="out_t")
                nc.vector.tensor_add(out_t[:], tr[:], h_sb[:, r, :])
                nc.sync.dma_start(t["out"][r * 128:(r + 1) * 128, :], out_t[:])


def build(flags):
    key = ("v2", flags)
    if key in _CACHE:
        return _CACHE[key]
    nc = bacc.Bacc("TRN2", target_bir_lowering=False, debug=False)
    t = _declare(nc)
    with tile.TileContext(nc) as tc:
        with ExitStack() as ctx:
            _emit(ctx, tc, t, flags)
    nc.compile()
    _CACHE[key] = nc
    return nc


def prep_core_inputs(inputs, core):
    """Host-side slicing + weight folding for one core."""
    b = core // 4
    g0 = (core % 4) * NBLK
    r0 = g0 * BLK

    f = lambda k: np.asarray(inputs[k], np.float32)
    ln_w, ln_b = f("ln_w"), f("ln_b")
    sc = 1.0 / np.sqrt(CH)

    def fold(w, scale=1.0):
        return ln_w[:, None] * np.asarray(w, np.float32) * scale

    def foldb(w, scale=1.0):
        return (ln_b @ np.asarray(w, np.float32)) * scale

    Wkv = f("Wkv")
    wq_h, bq_h = fold(inputs["Wq"], sc), foldb(inputs["Wq"], sc)
    wk_h, bk_h = fold(Wkv[:, :CS]), foldb(Wkv[:, :CS])
    wv_h, bv_h = fold(Wkv[:, CS:]), foldb(Wkv[:, CS:])
    wg_h, bg_h = fold(inputs["Wgate"]), foldb(inputs["Wgate"])
    if np.any(bv_h) or np.any(bg_h):
        raise NotImplementedError("nonzero folded v/gate bias unsupported")

    cw = f("adaln_cond_w")
    wada_h = np.concatenate(
        [cw[:, None] * f("W_ada_gate"), cw[:, None] * f("W_ada_bias"),
         cw[:, None] * f("W_tgate")], axis=1)
    bada_h = np.concatenate(
        [f("b_ada_gate"), np.zeros(CS, np.float32), f("b_tgate")]).astype(np.float32)

    wbias = f("bias_ln_w")[:, None] * f("Wbias")      # [128, 8]
    svec_h = (-wbias.sum(0)).astype(np.float32)       # minus S
    wbs_h = np.zeros((CZ, 64), np.float32)
    wbs_h[:, :H] = wbias
    wbs_h[:, 8] = 1.0 / CZ       # sum column directly produces the mean
    wbs_h[:, 32 + 9] = 1.0 / CZ  # z^2 pass accumulates E[z^2] into row 9

    def ktile(w, kt):
        w = np.asarray(w, np.float32)
        return np.ascontiguousarray(
            w.reshape(kt, 128, w.shape[1]).transpose(1, 0, 2)).astype(BF)

    # framepair: [16, 64(i), 64(j), 128] -> [16, 128, 4096] bf16, pairs
    # enumerated j-major so the kernel's reshape yields bias^T [j, m, i]
    fp = np.asarray(inputs["framepair_embed"][b, g0:g0 + NBLK], np.float32)
    zT = np.ascontiguousarray(fp.transpose(0, 3, 2, 1).reshape(
        NBLK, CZ, BLK * BLK)).astype(BF)

    idx = np.asarray(inputs["rigids_to_res_idx"][b, r0:r0 + NT]).astype(np.int16)
    idx_w = np.empty((128, NT // 16), np.int16)
    for p in range(16):
        idx_w[p] = idx[p::16]
    idx_w[16:] = np.tile(idx_w[:16], (7, 1))

    return {
        "re": np.ascontiguousarray(inputs["rigids_embed"][b, r0:r0 + NT]).astype(np.float32),
        "zT": zT,
        "s": np.ascontiguousarray(inputs["s"][b]).astype(np.float32),
        "idx": idx_w,
        "wq": ktile(wq_h, 4), "wk": ktile(wk_h, 4), "wv": ktile(wv_h, 4),
        "wg": ktile(wg_h, 4), "wout": ktile(inputs["Wout"], 4),
        "w1": ktile(inputs["W1"], 4), "w2": ktile(inputs["W2"], 4),
        "wb": ktile(inputs["Wb"], 8), "wada": ktile(wada_h, 3),
        "wbs": wbs_h.astype(BF), "svec": svec_h,
        "bq": np.ascontiguousarray(bq_h.reshape(4, 128).T),
        "bk": np.ascontiguousarray(bk_h.reshape(4, 128).T),
        "bada": bada_h,
    }, (bool(np.any(bq_h)), bool(np.any(bk_h)), bool(np.any(f("b_ada_gate"))),
        False, bool(np.any(f("b_tgate"))))


def kernel(**inputs):
    mask = np.asarray(inputs["rigids_mask"])
    if not np.all(mask == 1.0):
        print("WARNING: rigids_mask not all ones; kernel assumes ones", file=sys.stderr)

    in_maps, flags = [], None
    for core in range(NCORES):
        m, flags = prep_core_inputs(inputs, core)
        in_maps.append(m)

    nc = build(flags)
    res = run_bass_kernel_spmd(nc, in_maps, core_ids=list(range(NCORES)))

    out = np.empty((B, N, CS), np.float32)
    for core in range(NCORES):
        b = core // 4
        r0 = (core % 4) * NT
        out[b, r0:r0 + NT] = res.results[core]["out"]
    return out

